# revision 49
# baseline (speedup 1.0000x reference)
"""Trainium2 Bass kernel for nn_Discriminator (AdderNet CNN, 5 layers).

Per core (batch-sharded 256/8=32):
  adder2d(x,W) = -sum_d |p_d - w_d| = -S1 + SW + 2*M2
      S1 = sum_d p_d   (PE matmul, block-ones lhsT = -1.0, shared by all co)
      SW = sum_d w_d   (host constant, folded into ACT copy bias)
      M2 = sum_d min(p_d - w_d, 0)
           (DVE tensor_scalar (subtract,min), d on partitions, per-partition
            weight scalar; reduced over d by PE matmul with sliding one-hot
            lhsT = +2.0 into the psum row of the output channel)
  Training-mode BN: per-channel sum/sumsq via ACT accum_out, folded across
  psum rows by one-hot matmul, AllReduce [C,2] across 8 cores, scale/bias on
  device (sqrt + Newton + reciprocal), applied fused with LeakyReLU (Prelu).
  Layer-5 ends with Sigmoid. Patches for L2-L5 are strided ACT copies from
  zero-padded activation buffers; L1 patches (Ci=1) are im2col'd on host.
"""
import numpy as np
import ml_dtypes

NCORES = 8
NPC = 32
EPS = 1e-5
SLOPE = 0.2
BF = ml_dtypes.bfloat16

_cache = {}


def _install_bir_fix():
    """walrus workaround: ISA allows 1 sync-wait per instruction (2 for
    EventSemaphore); hoist excess waits onto injected EventSemaphores."""
    import orjson
    import concourse.bass_utils as bu
    import concourse.bass2jax as b2j

    if getattr(bu.compile_bir_kernel, "_waitfix", False):
        return

    def _fix(bir_json):
        bir = orjson.loads(bir_json)
        mods = bir.get("modules") or [bir]
        n = 0
        changed = False
        for mod in mods:
            for fn in mod.get("functions", []):
                for blk in fn.get("blocks", []):
                    out = []
                    for ins in blk.get("instructions", []):
                        cap = 2 if ins.get("opcode") == "EventSemaphore" else 1
                        waits = ins.get("sync_info", {}).get("on_wait", [])
                        if len(waits) > cap:
                            changed = True
                            for w in waits[:-cap]:
                                n += 1
                                out.append({
                                    "engine": ins["engine"], "ins": [], "outs": [],
                                    "name": f"I-waitfix-{n}",
                                    "opcode": "EventSemaphore",
                                    "sync_info": {"on_update": [], "on_wait": [w]},
                                    **({"debug": ins["debug"]} if "debug" in ins else {}),
                                })
                            ins["sync_info"]["on_wait"] = waits[-cap:]
                        out.append(ins)
                    blk["instructions"] = out
        return orjson.dumps(bir) if changed else bir_json

    orig = bu.compile_bir_kernel

    def wrapped(bir_json, tmpdir, neff_name="file.neff"):
        return orig(_fix(bir_json), tmpdir, neff_name)

    wrapped._waitfix = True
    bu.compile_bir_kernel = wrapped
    b2j.compile_bir_kernel = wrapped

    hook_orig = b2j.neuronx_cc_hook

    def hook_logged(*a, **k):
        try:
            return hook_orig(*a, **k)
        except BaseException:
            import traceback
            with open("/tmp/hook_err.log", "a") as f:
                f.write("=== neuronx_cc_hook failed ===\n")
                traceback.print_exc(file=f)
            raise

    b2j.neuronx_cc_hook = hook_logged


# layer geometry; d-order (kh, kw, ci); positions q = (n*Ho + ho)*Wo + wo
LCFG = {
    2: dict(Ci=16, Co=32, K=4, Ho=32, dparts=[128, 128], rowmod=32,
            npg=4, n_chunks=16, ck_n=2, rawW=8192, nst=16),
    3: dict(Ci=32, Co=64, K=3, Ho=16, dparts=[128, 128, 32], rowmod=64,
            npg=8, n_chunks=4, ck_n=8, rawW=4096, nst=8),
    4: dict(Ci=64, Co=128, K=4, Ho=8, dparts=[128] * 8, rowmod=128,
            npg=16, n_chunks=1, ck_n=32, rawW=2048, nst=4),
    5: dict(Ci=128, Co=1, K=4, Ho=4, dparts=[128] * 16, rowmod=128,
            npg=32, n_chunks=1, ck_n=32, rawW=512, nst=1),
}
# engine assignment pattern for min-producers and patch copies: B=DVE bf16
# (plain bf16 matmuls, PE absorbs 4x column cost), D=DVE fp8 (DoubleRow),
# A=ACT relu (negated, DoubleRow), P=Pool fp8 (DoubleRow). Weights from an LP
# equalizing DVE/ACT/Pool/PE busy per phase.


def _make_pat(weights, n):
    tot = sum(w for _, w in weights)
    acc = {e: 0.0 for e, _ in weights}
    out = []
    for _ in range(n):
        for e, w in weights:
            acc[e] += w / tot
        pick = max(acc, key=acc.get)
        acc[pick] -= 1.0
        out.append(pick)
    return "".join(out)


PAT = _make_pat([("D", 17), ("A", 9), ("P", 6)], 32)
_PATB = _make_pat([("D", 33), ("A", 20), ("P", 12)], 65)
PATL = {2: PAT, 3: _PATB, 4: _PATB, 5: PAT}
CNT = {1: 256 * 64 * 64, 2: 256 * 32 * 32, 3: 256 * 16 * 16,
       4: 256 * 8 * 8, 5: 256 * 4 * 4}
NCH = {1: 16, 2: 32, 3: 64, 4: 128, 5: 1}


def _build(taps=()):
    import contextlib
    import concourse.bass as bass
    import concourse.mybir as mybir
    from concourse.tile import TileContext

    F32 = mybir.dt.float32
    BF16 = mybir.dt.bfloat16
    FP8 = mybir.dt.float8e4
    A = mybir.AluOpType
    AF = mybir.ActivationFunctionType
    AX = mybir.AxisListType
    PM = mybir.MatmulPerfMode

    nc = bass.Bass(num_devices=NCORES)

    p1_d = nc.dram_tensor("p1", [128, 16384], BF16, kind="ExternalInput")
    w1rep_d = nc.dram_tensor("w1rep", [128, 16], F32, kind="ExternalInput")
    sw1_d = nc.dram_tensor("sw1", [128, 1], F32, kind="ExternalInput")
    wsc_cols = {2: 128, 3: 192, 4: 1024, 5: 16}
    wsc_d = {l: nc.dram_tensor(f"w{l}sc", [128, wsc_cols[l]], F32, kind="ExternalInput")
             for l in (2, 3, 4, 5)}
    w3c_d = nc.dram_tensor("w3c", [128, 16], F32, kind="ExternalInput")
    swb_d = {l: nc.dram_tensor(f"sw{l}", [128 if l < 5 else 1, 1], F32, kind="ExternalInput")
             for l in (2, 3, 4, 5)}
    gb_d = {l: nc.dram_tensor(f"gb{l}", [NCH[l], 2], F32, kind="ExternalInput")
            for l in (1, 2, 3, 4, 5)}
    out_d = nc.dram_tensor("out", [1, 512], F32, kind="ExternalOutput")
    tap_d = {}
    for t in taps:
        shp = {"raw1": [128, 16384], "raw2": [128, 8192], "raw3": [128, 4096],
               "raw4": [128, 2048], "raw5": [1, 512]}[t]
        tap_d[t] = nc.dram_tensor("tap_" + t, shp, F32 if t == "raw5" else BF16,
                                  kind="ExternalOutput")

    cc_in = {l: nc.dram_tensor(f"cci{l}", [NCH[l], 2], F32, kind="Internal")
             for l in (1, 2, 3, 4, 5)}
    cc_out = {l: nc.dram_tensor(f"cco{l}", [NCH[l], 2], F32, kind="Internal",
                                addr_space="Shared")
              for l in (1, 2, 3, 4, 5)}

    # inline constants
    ohg1 = np.zeros((128, 256), BF)
    bd16 = np.zeros((128, 128), BF)
    for k in range(128):
        ohg1[k, 128 + 16 * (k // 16)] = 2.0
        bd16[k, 16 * (k // 16):16 * (k // 16) + 16] = -1.0
    oh2b = np.zeros((128, 256), BF); oh2b[:, 128] = BF(2.0)
    on32 = np.zeros((128, 256), BF); on32[:, 128:160] = BF(-1.0)
    on64 = np.zeros((128, 256), BF); on64[:, 128:192] = BF(-1.0)
    neg1 = np.full((128, 128), -1.0, BF)
    onl2 = np.zeros((128, 256), BF)
    ohl2 = np.zeros((128, 256), BF)
    for k in range(128):
        par = (k % 32) // 16
        ohl2[k, 128 + 2 * par] = BF(2.0)
        for c in range(32):
            onl2[k, 128 + 4 * c + 2 * par] = BF(-1.0)
    # fp8 DoubleRow one-hots [128, 2, 256]; sign -2 for ACT-produced (negated)
    # planes. ohdr: plain row=slide; oh3c: L3-dt2 8-co packing row = slide +
    # 8*(2*(p//32)+i); ohl2d: L2 row = slide + 2*((p%32)//16)
    F8 = ml_dtypes.float8_e4m3fn
    ohdr = {}
    for sgn, sv in (("p", 2.0), ("m", -2.0)):
        a = np.zeros((128, 2, 256), F8); a[:, :, 128] = F8(sv)
        ohdr["dr" + sgn] = a
        a = np.zeros((128, 2, 256), F8)
        for p in range(128):
            for i in range(2):
                a[p, i, 128 + 8 * (2 * (p // 32) + i)] = F8(sv)
        ohdr["3c" + sgn] = a
        a = np.zeros((128, 2, 256), F8)
        for p in range(128):
            a[p, :, 128 + 2 * ((p % 32) // 16)] = F8(sv)
        ohdr["l2" + sgn] = a
    # sfm folds raw/psum-layout stats to channels; repm broadcasts channel
    # coeffs to PAD-buffer partition layout (channel = p % C for all layers)
    sfm, repm = {}, {}
    for l, C in ((1, 16), (2, 32), (3, 64)):
        m = np.zeros((128, C), np.float32)
        r = np.zeros((C, 128), np.float32)
        for k in range(128):
            m[k, (k // 4) if l == 2 else (k % C)] = 1.0
            r[k % C, k] = 1.0
        sfm[l], repm[l] = m, r
    inl = lambda nm, a: nc.inline_tensor(np.ascontiguousarray(a), name=nm)
    ohg1_t, bd16_t = inl("c_ohg1", ohg1), inl("c_bd16", bd16)
    oh2b_t = inl("c_oh2b", oh2b)
    on32_t = inl("c_on32", on32)
    on64_t, neg1_t = inl("c_on64", on64), inl("c_neg1", neg1)
    onl2_t = inl("c_onl2", onl2)
    ohl2_t = inl("c_ohl2", ohl2)
    ohdr_t = {k: inl("c_oh" + k, v) for k, v in ohdr.items()}
    sf_t = {l: inl(f"c_sf{l}", sfm[l]) for l in sfm}
    rep_t = {l: inl(f"c_rep{l}", repm[l]) for l in repm}

    with TileContext(nc) as tc:
        with contextlib.ExitStack() as ctx:
            cp = ctx.enter_context(tc.tile_pool(name="consts", bufs=1))
            hp = ctx.enter_context(tc.tile_pool(name="hpads", bufs=1))
            sp = ctx.enter_context(tc.tile_pool(name="small", bufs=1))
            scratch = ctx.enter_context(tc.tile_pool(name="scratch", bufs=2))
            ps = ctx.enter_context(tc.tile_pool(name="psum", bufs=6, space="PSUM"))
            ps2 = ctx.enter_context(tc.tile_pool(name="psum2", bufs=2, space="PSUM"))

            def load_const(tag, dram, shape, dtype):
                t = cp.tile(shape, dtype, name=tag, tag=tag)
                nc.sync.dma_start(t[:], dram[:])
                return t

            ohg1_s = load_const("ohg1", ohg1_t, [128, 256], BF16)
            bd16_s = load_const("bd16", bd16_t, [128, 128], BF16)
            oh2b_s = load_const("oh2b", oh2b_t, [128, 256], BF16)
            on32_s = load_const("on32", on32_t, [128, 256], BF16)
            on64_s = load_const("on64", on64_t, [128, 256], BF16)
            neg1_s = load_const("neg1", neg1_t, [128, 128], BF16)
            onl2_s = load_const("onl2", onl2_t, [128, 256], BF16)
            ohl2_s = load_const("ohl2", ohl2_t, [128, 256], BF16)
            ohdr_s = {k: load_const("oh" + k, t, [128, 2, 256], FP8)
                      for k, t in ohdr_t.items()}
            sf_s = {l: load_const(f"sf{l}", sf_t[l], [128, NCH[l]], F32) for l in sfm}
            rep_s = {l: load_const(f"rep{l}", rep_t[l], [NCH[l], 128], F32) for l in repm}
            w1rep_s = load_const("w1rep", w1rep_d, [128, 16], F32)
            sw1_s = load_const("sw1", sw1_d, [128, 1], F32)
            wsc_s = {l: load_const(f"wsc{l}", wsc_d[l], [128, wsc_cols[l]], F32)
                     for l in (2, 3, 4, 5)}
            w3c_s = load_const("w3c", w3c_d, [128, 16], F32)
            swb_s = {l: load_const(f"swb{l}", swb_d[l], [128 if l < 5 else 1, 1], F32)
                     for l in (2, 3, 4, 5)}
            gb_s = {l: load_const(f"gb{l}", gb_d[l], [NCH[l], 2], F32)
                    for l in (1, 2, 3, 4, 5)}

            h1p = hp.tile([128, 4, 66, 66], BF16, name="h1p", tag="h1p")
            h2p = hp.tile([128, 8, 34, 34], BF16, name="h2p", tag="h2p")
            h3p = hp.tile([128, 16, 18, 18], BF16, name="h3p", tag="h3p")
            h4p = hp.tile([128, 32, 10, 10], BF16, name="h4p", tag="h4p")
            for t in (h1p, h2p, h3p, h4p):
                nc.gpsimd.memset(t[:], 0.0)

            st_s = {l: sp.tile([128 if l < 5 else 1, LCFG[l]["nst"] if l > 1 else 32],
                               F32, name=f"sts{l}", tag=f"sts{l}") for l in (1, 2, 3, 4, 5)}
            st_q = {l: sp.tile([128 if l < 5 else 1, LCFG[l]["nst"] if l > 1 else 32],
                               F32, name=f"stq{l}", tag=f"stq{l}") for l in (1, 2, 3, 4, 5)}

            def bn_coeffs(l):
                C = NCH[l]
                R = 128 if l < 5 else 1
                stf = sp.tile([R, 2], F32, name=f"stf{l}", tag=f"stf{l}")
                nc.vector.tensor_reduce(stf[:, 0:1], st_s[l][:], AX.X, A.add)
                nc.vector.tensor_reduce(stf[:, 1:2], st_q[l][:], AX.X, A.add)
                stc = sp.tile([C, 2], F32, name=f"stc{l}", tag=f"stc{l}")
                if l in sf_s:
                    psf = ps2.tile([C, 2], F32, name="paux", tag="paux")
                    nc.tensor.matmul(psf[:], sf_s[l][:], stf[:], start=True, stop=True)
                    nc.scalar.copy(stc[:], psf[:])
                else:
                    nc.vector.tensor_copy(stc[:], stf[:])
                nc.sync.dma_start(cc_in[l][:], stc[:])
                nc.gpsimd.collective_compute(
                    "AllReduce", A.add, replica_groups=[list(range(NCORES))],
                    ins=[cc_in[l][:]], outs=[cc_out[l][:]])
                nb = sp.tile([R, 4], F32, name=f"nb{l}", tag=f"nb{l}")
                if l in rep_s:
                    rr = sp.tile([C, 4], F32, name=f"rr{l}", tag=f"rr{l}")
                    nc.sync.dma_start(rr[:, 0:2], cc_out[l][:])
                    nc.vector.tensor_copy(rr[:, 2:4], gb_s[l][:])
                    prr = ps2.tile([128, 4], F32, name="paux", tag="paux")
                    nc.tensor.matmul(prr[:], rep_s[l][:], rr[:], start=True, stop=True)
                    nc.scalar.copy(nb[:], prr[:])
                else:
                    nc.sync.dma_start(nb[:, 0:2], cc_out[l][:])
                    nc.vector.tensor_copy(nb[:, 2:4], gb_s[l][:])
                ic = 1.0 / CNT[l]
                mS = sp.tile([R, 1], F32, name=f"mS{l}", tag=f"mS{l}")
                v = sp.tile([R, 1], F32, name=f"v{l}", tag=f"v{l}")
                nc.vector.tensor_scalar_mul(mS[:], nb[:, 0:1], ic)
                nc.vector.tensor_tensor(v[:], mS[:], mS[:], A.mult)
                mQ = sp.tile([R, 1], F32, name=f"mQ{l}", tag=f"mQ{l}")
                nc.vector.tensor_scalar_mul(mQ[:], nb[:, 1:2], ic)
                nc.vector.tensor_tensor(v[:], mQ[:], v[:], A.subtract)
                nc.vector.tensor_scalar_add(v[:], v[:], EPS)
                y0 = sp.tile([R, 1], F32, name=f"y0{l}", tag=f"y0{l}")
                nc.scalar.activation(y0[:], v[:], AF.Sqrt)
                r0 = sp.tile([R, 1], F32, name=f"r0{l}", tag=f"r0{l}")
                nc.vector.reciprocal(r0[:], y0[:])
                t0 = sp.tile([R, 1], F32, name=f"t0{l}", tag=f"t0{l}")
                nc.vector.tensor_tensor(t0[:], v[:], r0[:], A.mult)
                nc.vector.tensor_tensor(t0[:], y0[:], t0[:], A.add)
                nc.vector.tensor_scalar_mul(t0[:], t0[:], 0.5)
                rsq = sp.tile([R, 1], F32, name=f"rsq{l}", tag=f"rsq{l}")
                nc.vector.reciprocal(rsq[:], t0[:])
                a = sp.tile([R, 1], F32, name=f"a{l}", tag=f"a{l}")
                nc.vector.tensor_tensor(a[:], nb[:, 2:3], rsq[:], A.mult)
                c = sp.tile([R, 1], F32, name=f"c{l}", tag=f"c{l}")
                nc.vector.tensor_tensor(c[:], mS[:], a[:], A.mult)
                nc.vector.tensor_tensor(c[:], nb[:, 3:4], c[:], A.subtract)
                return a, c

            # ---------------- Layer 1 ----------------
            with tc.tile_pool(name="l1raw", bufs=1) as rp1, \
                 tc.tile_pool(name="l1p", bufs=2) as pp1, \
                 tc.tile_pool(name="l1d", bufs=3) as dp1:
                raw1 = rp1.tile([128, 16384], BF16, name="raw1", tag="raw1")
                for ch in range(8):
                    p1c = pp1.tile([128, 2048], BF16, name="p1c", tag="p1c")
                    nc.sync.dma_start(p1c[:], p1_d[:, ch * 2048:(ch + 1) * 2048])
                    pts = [ps.tile([128, 512], F32, name="pmain", tag="pmain") for _ in range(4)]
                    for tt in range(4):
                        nc.tensor.matmul(pts[tt][:], bd16_s[:],
                                         p1c[:, tt * 512:(tt + 1) * 512],
                                         start=True, stop=False)
                    for co in range(16):
                        d1 = dp1.tile([128, 2048], BF16, name="d1", tag="d1")
                        nc.vector.tensor_scalar(d1[:], p1c[:], w1rep_s[:, co:co + 1],
                                                0.0, A.subtract, A.min)
                        for tt in range(4):
                            nc.tensor.matmul(pts[tt][:],
                                             ohg1_s[:, 128 - co:256 - co],
                                             d1[:, tt * 512:(tt + 1) * 512],
                                             start=False, stop=(co == 15))
                    for tt in range(4):
                        t = ch * 4 + tt
                        nc.scalar.activation(raw1[:, t * 512:(t + 1) * 512], pts[tt][:],
                                             AF.Identity, bias=sw1_s[:, 0:1],
                                             accum_out=st_s[1][:, t:t + 1])
                        sq = scratch.tile([128, 512], F32, name="sq1", tag="sq1")
                        nc.scalar.activation(sq[:], raw1[:, t * 512:(t + 1) * 512],
                                             AF.Square, accum_out=st_q[1][:, t:t + 1])
                        # pre-BN pad writes, overlapped with later chunks
                        base = 32 * (t // 8) + 16 * (t % 2)
                        for g in range(8):
                            nc.sync.dma_start(
                                h1p[base:base + 16, (t % 8) // 2,
                                    1 + 8 * g:9 + 8 * g, 1:65],
                                raw1[16 * g:16 * g + 16, t * 512:(t + 1) * 512])

                a1, c1 = bn_coeffs(1)
                nc.scalar.activation(h1p[:, :, 1:65, 1:65], h1p[:, :, 1:65, 1:65],
                                     AF.Prelu, bias=c1[:, 0:1], scale=a1[:, 0:1],
                                     alpha=SLOPE)

            # ---------------- Layers 2-5 ----------------
            def emit_min(eng, dst, src, wcol):
                # dst: fp8 plane [128, npos]. D/P produce min(p-w, 0);
                # A produces relu(w-p) = -min(p-w,0) (pair with -2 one-hot)
                if eng == "D":
                    nc.vector.tensor_scalar(dst, src, wcol, 0.0, A.subtract, A.min)
                elif eng == "P":
                    nc.gpsimd.tensor_scalar(dst, src, wcol, 0.0, A.subtract, A.min)
                else:
                    nc.scalar.activation(dst, src, AF.Relu, bias=wcol, scale=-1.0)

            def emit_copy(eng, dst, src):
                if eng in ("D", "B"):
                    nc.vector.tensor_copy(dst, src)
                elif eng == "P":
                    nc.gpsimd.tensor_copy(dst, src)
                else:
                    nc.scalar.copy(dst, src)

            def run_layer(l, src_pad, raw_t, pool_p, pool_d, pool_db, pad_emit=None):
                cfg = LCFG[l]
                Ci, Co, K, Ho = cfg["Ci"], cfg["Co"], cfg["K"], cfg["Ho"]
                Wo, rowmod, dparts, npg = Ho, cfg["rowmod"], cfg["dparts"], cfg["npg"]
                ck_n = cfg["ck_n"]
                npos = ck_n * Ho * Wo
                nsub = npos // 512
                nblk = 128 // rowmod
                n_pt = max(1, nsub // nblk)
                nfull = sum(1 for d in dparts if d == 128)
                npairs = nfull // 2
                has_tail = (l == 3)
                onesb = {32: on32_s, 64: on64_s, 128: neg1_s}[rowmod]
                unit = l * 5
                for ch in range(cfg["n_chunks"]):
                    ptiles = []
                    for dt in range(nfull):
                        nk = 128 // Ci
                        pt_ = pool_p.tile([128, ck_n, Ho, Wo], BF16, name=f"p{l}_{dt}", tag=f"p{l}_{dt}")
                        for kk in range(nk):
                            khkw = dt * nk + kk
                            kh, kw = khkw // K, khkw % K
                            n0 = ch * ck_n
                            for gg in range(n0 // npg, (n0 + ck_n - 1) // npg + 1):
                                na = max(n0, gg * npg)
                                nb_ = min(n0 + ck_n, (gg + 1) * npg)
                                if na >= nb_:
                                    continue
                                eng = PATL[l][unit % len(PATL[l])]
                                unit += 1
                                emit_copy(
                                    eng,
                                    pt_[kk * Ci:(kk + 1) * Ci, na - n0:nb_ - n0, :, :],
                                    src_pad[Ci * gg:Ci * gg + Ci,
                                            na - gg * npg:nb_ - gg * npg,
                                            kh:kh + 2 * Ho - 1:2,
                                            kw:kw + 2 * Wo - 1:2])
                        ptiles.append(pt_)
                    pt3 = None
                    if has_tail:
                        # tail dtile (kh=kw=2, 32 d) replicated to 4 groups
                        pt3 = pool_p.tile([128, ck_n, Ho, Wo], BF16,
                                          name=f"p{l}_t", tag=f"p{l}_t")
                        gg = ch  # npg == ck_n == 8 for L3
                        for rep in range(4):
                            eng = PATL[l][unit % len(PATL[l])]
                            unit += 1
                            emit_copy(
                                eng,
                                pt3[rep * 32:rep * 32 + 32, :, :, :],
                                src_pad[32 * gg:32 * gg + 32, :,
                                        2:2 + 2 * Ho - 1:2, 2:2 + 2 * Wo - 1:2])
                    pts = [ps.tile([128, 512], F32, name="pmain", tag="pmain") for _ in range(n_pt)]
                    first = [True] * n_pt
                    for dt in range(nfull):
                        pvf = ptiles[dt][:].rearrange("p a b c -> p (a b c)")
                        for s in range(nsub):
                            tt, j = s // nblk, s % nblk
                            lhs = (onesb[:, :] if rowmod == 128 else
                                   onesb[:, 128 - rowmod * j:256 - rowmod * j])
                            nc.tensor.matmul(pts[tt][:], lhs,
                                             pvf[:, s * 512:(s + 1) * 512],
                                             start=first[tt], stop=False)
                            first[tt] = False
                    if has_tail:
                        pvf = pt3[:].rearrange("p a b c -> p (a b c)")
                        for s in range(nsub):
                            tt, j = s // nblk, s % nblk
                            nc.tensor.matmul(pts[tt][:],
                                             onesb[0:32, 128 - rowmod * j:256 - rowmod * j],
                                             pvf[0:32, s * 512:(s + 1) * 512],
                                             start=False, stop=False)
                    if l == 5:
                        # tiny layer: bf16 (better accuracy, negligible cost)
                        for dt in range(16):
                            dl = pool_d.tile([128, 32, 4, 4], BF16, name="dl5b", tag="dl5b")
                            nc.vector.tensor_scalar(
                                dl[:], ptiles[dt][:],
                                wsc_s[5][:, dt:dt + 1], 0.0, A.subtract, A.min)
                            nc.tensor.matmul(
                                pts[0][:], oh2b_s[:, 128:256],
                                dl[:].rearrange("p a b c -> p (a b c)"),
                                start=False, stop=(dt == 15))
                        bunits, dunits = [], []
                    else:
                        pat = PATL[l]
                        units = [("m", co, pr) for co in range(Co) for pr in range(npairs)]
                        if has_tail:
                            units += [("t", c0, 0) for c0 in range(8)]
                        engs = []
                        for kind, _, _ in units:
                            e = pat[unit % len(pat)]
                            unit += 1
                            if kind == "t" and e == "B":
                                e = "D"
                            engs.append(e)
                        bunits = [u for u, e in zip(units, engs) if e == "B"]
                        dunits = [(u, e) for u, e in zip(units, engs) if e != "B"]
                    # pass 1: bf16 units (normal matmuls). All emitted before any
                    # DoubleRow matmul: the PE must not alternate perf modes
                    # within a psum accumulation group (corrupts results).
                    for kind, co, pr in bunits:
                        dl = pool_db.tile([128, 2, npos], BF16,
                                          name=f"dlb{l}", tag=f"dlb{l}")
                        for i in (0, 1):
                            dt = 2 * pr + i
                            emit_min("D", dl[:, i, :],
                                     ptiles[dt][:].rearrange("p a b c -> p (a b c)"),
                                     wsc_s[l][:, dt * Co + co:dt * Co + co + 1])
                        for s in range(nsub):
                            tt, j = s // nblk, s % nblk
                            r = (rowmod * j + co) if rowmod < 128 else co
                            for i in (0, 1):
                                nc.tensor.matmul(
                                    pts[tt][:], oh2b_s[:, 128 - r:256 - r],
                                    dl[:, i, s * 512:(s + 1) * 512],
                                    start=False, stop=False)
                    # pass 2: fp8 DoubleRow units
                    for ui, ((kind, a0, a1), eng) in enumerate(dunits):
                        lastu = (ui == len(dunits) - 1)
                        dl = pool_d.tile([128, 2, npos], FP8, name=f"dl{l}", tag=f"dl{l}")
                        if kind == "m":
                            co = a0
                            for i in (0, 1):
                                dt = 2 * a1 + i
                                emit_min(eng, dl[:, i, :],
                                         ptiles[dt][:].rearrange("p a b c -> p (a b c)"),
                                         wsc_s[l][:, dt * Co + co:dt * Co + co + 1])
                            oh = ohdr_s["dr" + ("m" if eng == "A" else "p")]
                            rbase = a0
                        else:
                            pvf3 = pt3[:].rearrange("p a b c -> p (a b c)")
                            for i in (0, 1):
                                emit_min(eng, dl[:, i, :], pvf3,
                                         w3c_s[:, i * 8 + a0:i * 8 + a0 + 1])
                            oh = ohdr_s["3c" + ("m" if eng == "A" else "p")]
                            rbase = a0
                        for s in range(nsub):
                            tt, j = s // nblk, s % nblk
                            r = (rowmod * j + rbase) if rowmod < 128 else rbase
                            for half in (0, 1):
                                off = s * 512 + half * 256
                                if Co == 1:
                                    outp = pts[tt][0:1, half * 256:half * 256 + 256]
                                    lhsT = oh[:, :, 128:129]
                                else:
                                    outp = pts[tt][:, half * 256:half * 256 + 256]
                                    lhsT = oh[:, :, 128 - r:256 - r]
                                nc.tensor.matmul(
                                    outp, lhsT, dl[:, :, off:off + 256],
                                    start=False,
                                    stop=(lastu and j == nblk - 1 and half == 1),
                                    perf_mode=PM.DoubleRow)
                    R = 128 if l < 5 else 1
                    for tt in range(n_pt):
                        t = ch * n_pt + tt
                        nc.scalar.activation(
                            raw_t[0:R, t * 512:(t + 1) * 512], pts[tt][0:R, :],
                            AF.Identity, bias=swb_s[l][:, 0:1],
                            accum_out=st_s[l][:, t:t + 1])
                        sq = scratch.tile([R, 512], F32, name=f"sq{l}", tag=f"sq{l}")
                        nc.scalar.activation(sq[:], raw_t[0:R, t * 512:(t + 1) * 512],
                                             AF.Square, accum_out=st_q[l][:, t:t + 1])
                        if pad_emit is not None:
                            pad_emit(t)

            # L2: partitions pack (k4, 2 images, 16 ci); chunk = 4 images
            # (2 dlo pairs); psum rows r = 4co + 2par + hh, tile per dlo
            with tc.tile_pool(name="l2raw", bufs=1) as rp2, \
                 tc.tile_pool(name="l2p", bufs=2) as pp2, \
                 tc.tile_pool(name="l2d", bufs=8) as dp2, \
                 tc.tile_pool(name="l2db", bufs=2) as dp2b:
                raw2 = rp2.tile([128, 8192], BF16, name="raw2", tag="raw2")
                unit = 7
                for ch in range(16):
                    ptiles = []
                    for dt in range(4):
                        pt_ = pp2.tile([128, 32, 32], BF16, name=f"p2_{dt}",
                                       tag=f"p2_{dt}")
                        for k4 in range(4):
                            khkw = dt * 4 + k4
                            kh, kw = khkw // 4, khkw % 4
                            eng = PATL[2][unit % len(PATL[2])]
                            unit += 1
                            emit_copy(
                                eng,
                                pt_[32 * k4:32 * k4 + 32, :, :],
                                h1p[32 * (ch // 4):32 * (ch // 4) + 32, ch % 4,
                                    kh:kh + 63:2, kw:kw + 63:2])
                        ptiles.append(pt_)
                    pt = ps.tile([128, 512], F32, name="pmain", tag="pmain")
                    first = True
                    for dt in range(4):
                        pvf = ptiles[dt][:].rearrange("p a b -> p (a b)")
                        for hh in (0, 1):
                            nc.tensor.matmul(
                                pt[:], onl2_s[:, 128 - hh:256 - hh],
                                pvf[:, 512 * hh:512 * hh + 512],
                                start=first, stop=False)
                            first = False
                    units2 = [(co, pr) for co in range(32) for pr in range(2)]
                    pat2 = PATL[2]
                    engs2 = []
                    for _ in units2:
                        engs2.append(pat2[unit % len(pat2)])
                        unit += 1
                    # pass 1: bf16 units (see run_layer: no perf-mode alternation)
                    for (co, pr), e in zip(units2, engs2):
                        if e != "B":
                            continue
                        dl = dp2b.tile([128, 2, 1024], BF16, name="dlb2", tag="dlb2")
                        for i in (0, 1):
                            dt = 2 * pr + i
                            emit_min("D", dl[:, i, :],
                                     ptiles[dt][:].rearrange("p a b -> p (a b)"),
                                     wsc_s[2][:, dt * 32 + co:dt * 32 + co + 1])
                        for hh in (0, 1):
                            r = 4 * co + hh
                            for i in (0, 1):
                                nc.tensor.matmul(
                                    pt[:], ohl2_s[:, 128 - r:256 - r],
                                    dl[:, i, hh * 512:hh * 512 + 512],
                                    start=False, stop=False)
                    dunits2 = [(u, e) for u, e in zip(units2, engs2) if e != "B"]
                    for ui, ((co, pr), eng) in enumerate(dunits2):
                        lastu = (ui == len(dunits2) - 1)
                        dl = dp2.tile([128, 2, 1024], FP8, name="dl2", tag="dl2")
                        for i in (0, 1):
                            dt = 2 * pr + i
                            emit_min(eng, dl[:, i, :],
                                     ptiles[dt][:].rearrange("p a b -> p (a b)"),
                                     wsc_s[2][:, dt * 32 + co:dt * 32 + co + 1])
                        oh = ohdr_s["l2" + ("m" if eng == "A" else "p")]
                        for hh in (0, 1):
                            r = 4 * co + hh
                            for half in (0, 1):
                                off = hh * 512 + half * 256
                                nc.tensor.matmul(
                                    pt[:, half * 256:half * 256 + 256],
                                    oh[:, :, 128 - r:256 - r],
                                    dl[:, :, off:off + 256],
                                    start=False,
                                    stop=(lastu and hh == 1 and half == 1),
                                    perf_mode=PM.DoubleRow)
                    nc.scalar.activation(raw2[:, ch * 512:(ch + 1) * 512], pt[:],
                                         AF.Identity, bias=swb_s[2][:, 0:1],
                                         accum_out=st_s[2][:, ch:ch + 1])
                    sq = scratch.tile([128, 512], F32, name="sq2", tag="sq2")
                    nc.scalar.activation(sq[:], raw2[:, ch * 512:(ch + 1) * 512],
                                         AF.Square, accum_out=st_q[2][:, ch:ch + 1])
                    for j in range(4):
                        pos0 = ch * 2048 + j * 512
                        n, hh = pos0 // 1024, (pos0 % 1024) // 512
                        nc.sync.dma_start(
                            h2p[32 * (n // 8):32 * (n // 8) + 32, n % 8,
                                1 + 16 * hh:17 + 16 * hh, 1:33],
                            raw2[j:128:4, ch * 512:(ch + 1) * 512])
                a2, c2 = bn_coeffs(2)
                nc.scalar.activation(h2p[:, :, 1:33, 1:33], h2p[:, :, 1:33, 1:33],
                                     AF.Prelu, bias=c2[:, 0:1], scale=a2[:, 0:1],
                                     alpha=SLOPE)

            # L3
            with tc.tile_pool(name="l3raw", bufs=1) as rp3, \
                 tc.tile_pool(name="l3p", bufs=2) as pp3, \
                 tc.tile_pool(name="l3d", bufs=8) as dp3, \
                 tc.tile_pool(name="l3db", bufs=2) as dp3b:
                raw3 = rp3.tile([128, 4096], BF16, name="raw3", tag="raw3")

                def pad3(t):
                    for j in range(2):
                        n = (t * 1024 + j * 512) // 256
                        for i in range(2):
                            nc.sync.dma_start(
                                h3p[64 * (n // 16):64 * (n // 16) + 64,
                                    n % 16 + i, 1:17, 1:17],
                                raw3[64 * j:64 * j + 64,
                                     t * 512 + i * 256:t * 512 + (i + 1) * 256])

                run_layer(3, h2p, raw3, pp3, dp3, dp3b, pad_emit=pad3)
                a3, c3 = bn_coeffs(3)
                nc.scalar.activation(h3p[:, :, 1:17, 1:17], h3p[:, :, 1:17, 1:17],
                                     AF.Prelu, bias=c3[:, 0:1], scale=a3[:, 0:1],
                                     alpha=SLOPE)

            # L4
            with tc.tile_pool(name="l4raw", bufs=1) as rp4, \
                 tc.tile_pool(name="l4p", bufs=1) as pp4, \
                 tc.tile_pool(name="l4d", bufs=8) as dp4, \
                 tc.tile_pool(name="l4db", bufs=2) as dp4b:
                raw4 = rp4.tile([128, 2048], BF16, name="raw4", tag="raw4")

                def pad4(t):
                    for i in range(8):
                        nc.sync.dma_start(
                            h4p[:, 8 * t + i, 1:9, 1:9],
                            raw4[:, t * 512 + i * 64:t * 512 + (i + 1) * 64])

                run_layer(4, h3p, raw4, pp4, dp4, dp4b, pad_emit=pad4)
                a4, c4 = bn_coeffs(4)
                nc.scalar.activation(h4p[:, :, 1:9, 1:9], h4p[:, :, 1:9, 1:9],
                                     AF.Prelu, bias=c4[:, 0:1], scale=a4[:, 0:1],
                                     alpha=SLOPE)

            # L5
            with tc.tile_pool(name="l5raw", bufs=1) as rp5, \
                 tc.tile_pool(name="l5p", bufs=1) as pp5, \
                 tc.tile_pool(name="l5d", bufs=8) as dp5, \
                 tc.tile_pool(name="l5db", bufs=2) as dp5b:
                raw5 = rp5.tile([1, 512], F32, name="raw5", tag="raw5")
                run_layer(5, h4p, raw5, pp5, dp5, dp5b)
                a5, c5 = bn_coeffs(5)
                out5 = sp.tile([1, 512], F32, name="out5", tag="out5")
                nc.scalar.activation(out5[:], raw5[:], AF.Sigmoid,
                                     bias=c5[:, 0:1], scale=a5[:, 0:1])
                if "raw5" in tap_d:
                    nc.sync.dma_start(tap_d["raw5"][:], raw5[:])
                nc.sync.dma_start(out_d[:], out5[:])

    return nc


def _host_prep(inputs):
    x = np.asarray(inputs["x"], np.float32)
    W = {l: np.asarray(inputs[f"W{l}"], np.float32) for l in (1, 2, 3, 4, 5)}
    g = {l: np.asarray(inputs[f"g{l}"], np.float32) for l in (1, 2, 3, 4, 5)}
    b = {l: np.asarray(inputs[f"b{l}"], np.float32) for l in (1, 2, 3, 4, 5)}

    W1f = W[1].reshape(16, 16)
    shared = {
        "w1rep": np.ascontiguousarray(np.tile(W1f.T, (8, 1)), np.float32),
        "sw1": np.ascontiguousarray(np.tile(W1f.sum(1), 8)[:, None], np.float32),
    }
    for l in (2, 3, 4, 5):
        Wd = W[l].transpose(2, 3, 1, 0).reshape(-1, W[l].shape[0])  # [D, Co]
        D, Co = Wd.shape
        if l == 2:
            wsc = np.zeros((128, 4 * 32), np.float32)
            for k in range(128):
                for dt in range(4):
                    khkw = dt * 4 + k // 32
                    wsc[k, dt * 32:(dt + 1) * 32] = Wd[khkw * 16 + (k % 16), :]
            shared["w2sc"] = wsc
        else:
            ndt = len(LCFG[l]["dparts"])
            Wp = np.zeros((ndt * 128, Co), np.float32)
            Wp[:D] = Wd
            shared[f"w{l}sc"] = np.ascontiguousarray(
                Wp.reshape(ndt, 128, Co).transpose(1, 0, 2).reshape(128, ndt * Co))
        if l == 3:
            w3c = np.zeros((128, 16), np.float32)
            for p in range(128):
                for i in range(2):
                    for c0 in range(8):
                        w3c[p, i * 8 + c0] = Wd[256 + p % 32,
                                                c0 + 8 * (2 * (p // 32) + i)]
            shared["w3c"] = w3c
        swl = Wd.sum(0)
        if l < 5:
            idx = (lambda k: k // 4) if l == 2 else (lambda k: k % Co)
            shared[f"sw{l}"] = np.asarray(
                [swl[idx(k)] for k in range(128)], np.float32)[:, None]
        else:
            shared[f"sw{l}"] = np.ascontiguousarray(swl[:, None], np.float32)
    for l in (1, 2, 3, 4, 5):
        shared[f"gb{l}"] = np.ascontiguousarray(
            np.stack([g[l].ravel(), b[l].ravel()], 1), np.float32)

    in_maps = []
    for c in range(NCORES):
        xs = x[c * NPC:(c + 1) * NPC, 0]
        xp = np.pad(xs, ((0, 0), (1, 1), (1, 1)))
        s = xp.strides
        win = np.lib.stride_tricks.as_strided(
            xp, (NPC, 64, 64, 4, 4), (s[0], 2 * s[1], 2 * s[2], s[1], s[2]))
        P1 = win.transpose(3, 4, 0, 1, 2).reshape(16, NPC * 4096)
        p1 = np.ascontiguousarray(
            P1.reshape(16, 32, 8, 512).transpose(2, 0, 1, 3).reshape(128, 16384),
            dtype=BF)
        m = dict(shared)
        m["p1"] = p1
        in_maps.append(m)
    return in_maps


def _run(inputs, taps=(), **kw):
    _install_bir_fix()
    from concourse.bass_utils import run_bass_kernel_spmd
    key = tuple(sorted(taps))
    if key not in _cache:
        _cache[key] = _build(taps)
    in_maps = _host_prep(inputs)
    return run_bass_kernel_spmd(_cache[key], in_maps, list(range(NCORES)), **kw)


def kernel(**inputs):
    res = _run(inputs)
    out = np.zeros((256, 1, 4, 4), np.float32)
    for c in range(NCORES):
        o = np.asarray(res.results[c]["out"], np.float32).reshape(NPC, 4, 4)
        out[c * NPC:(c + 1) * NPC, 0] = o
    return out



# revision 50
# speedup vs baseline: 1.4430x; 1.4430x over previous
"""Trainium2 Bass kernel for nn_Discriminator (AdderNet CNN, 5 layers).

Per core (batch-sharded 256/8=32):
  adder2d(x,W) = -sum_d |p_d - w_d| = -S1 + SW + 2*M2
      S1 = sum_d p_d   (PE matmul, block-ones lhsT = -1.0, shared by all co)
      SW = sum_d w_d   (host constant, folded into ACT copy bias)
      M2 = sum_d min(p_d - w_d, 0)
           (DVE tensor_scalar (subtract,min), d on partitions, per-partition
            weight scalar; reduced over d by PE matmul with sliding one-hot
            lhsT = +2.0 into the psum row of the output channel)
  Training-mode BN: per-channel sum/sumsq via ACT accum_out, folded across
  psum rows by one-hot matmul, AllReduce [C,2] across 8 cores, scale/bias on
  device (sqrt + Newton + reciprocal), applied fused with LeakyReLU (Prelu).
  Layer-5 ends with Sigmoid. Patches for L2-L5 are strided ACT copies from
  zero-padded activation buffers; L1 patches (Ci=1) are im2col'd on host.
"""
import numpy as np
import ml_dtypes

NCORES = 8
NPC = 32
EPS = 1e-5
SLOPE = 0.2
BF = ml_dtypes.bfloat16

_cache = {}


def _install_bir_fix():
    """walrus workaround: ISA allows 1 sync-wait per instruction (2 for
    EventSemaphore); hoist excess waits onto injected EventSemaphores."""
    import orjson
    import concourse.bass_utils as bu
    import concourse.bass2jax as b2j

    if getattr(bu.compile_bir_kernel, "_waitfix", False):
        return

    def _fix(bir_json):
        bir = orjson.loads(bir_json)
        mods = bir.get("modules") or [bir]
        n = 0
        changed = False
        for mod in mods:
            for fn in mod.get("functions", []):
                for blk in fn.get("blocks", []):
                    out = []
                    for ins in blk.get("instructions", []):
                        cap = 2 if ins.get("opcode") == "EventSemaphore" else 1
                        waits = ins.get("sync_info", {}).get("on_wait", [])
                        if len(waits) > cap:
                            changed = True
                            for w in waits[:-cap]:
                                n += 1
                                out.append({
                                    "engine": ins["engine"], "ins": [], "outs": [],
                                    "name": f"I-waitfix-{n}",
                                    "opcode": "EventSemaphore",
                                    "sync_info": {"on_update": [], "on_wait": [w]},
                                    **({"debug": ins["debug"]} if "debug" in ins else {}),
                                })
                            ins["sync_info"]["on_wait"] = waits[-cap:]
                        out.append(ins)
                    blk["instructions"] = out
        return orjson.dumps(bir) if changed else bir_json

    orig = bu.compile_bir_kernel

    def wrapped(bir_json, tmpdir, neff_name="file.neff"):
        return orig(_fix(bir_json), tmpdir, neff_name)

    wrapped._waitfix = True
    bu.compile_bir_kernel = wrapped
    b2j.compile_bir_kernel = wrapped

    hook_orig = b2j.neuronx_cc_hook

    def hook_logged(*a, **k):
        try:
            return hook_orig(*a, **k)
        except BaseException:
            import traceback
            with open("/tmp/hook_err.log", "a") as f:
                f.write("=== neuronx_cc_hook failed ===\n")
                traceback.print_exc(file=f)
            raise

    b2j.neuronx_cc_hook = hook_logged


# layer geometry; d-order (kh, kw, ci); positions q = (n*Ho + ho)*Wo + wo
LCFG = {
    2: dict(Ci=16, Co=32, K=4, Ho=32, dparts=[128, 128], rowmod=32,
            npg=4, n_chunks=16, ck_n=2, rawW=8192, nst=16),
    3: dict(Ci=32, Co=64, K=3, Ho=16, dparts=[128, 128, 32], rowmod=64,
            npg=8, n_chunks=4, ck_n=8, rawW=4096, nst=8),
    4: dict(Ci=64, Co=128, K=4, Ho=8, dparts=[128] * 8, rowmod=128,
            npg=16, n_chunks=1, ck_n=32, rawW=2048, nst=4),
    5: dict(Ci=128, Co=1, K=4, Ho=4, dparts=[128] * 16, rowmod=128,
            npg=32, n_chunks=1, ck_n=32, rawW=512, nst=1),
}
# engine assignment pattern for min-producers and patch copies: B=DVE bf16
# (plain bf16 matmuls, PE absorbs 4x column cost), D=DVE fp8 (DoubleRow),
# A=ACT relu (negated, DoubleRow), P=Pool fp8 (DoubleRow). Weights from an LP
# equalizing DVE/ACT/Pool/PE busy per phase.


def _make_pat(weights, n):
    tot = sum(w for _, w in weights)
    acc = {e: 0.0 for e, _ in weights}
    out = []
    for _ in range(n):
        for e, w in weights:
            acc[e] += w / tot
        pick = max(acc, key=acc.get)
        acc[pick] -= 1.0
        out.append(pick)
    return "".join(out)


PAT = _make_pat([("D", 17), ("A", 9), ("P", 6)], 32)
_PATB = _make_pat([("D", 33), ("A", 20), ("P", 12)], 65)
PATL = {2: PAT, 3: _PATB, 4: _PATB, 5: PAT}
CNT = {1: 256 * 64 * 64, 2: 256 * 32 * 32, 3: 256 * 16 * 16,
       4: 256 * 8 * 8, 5: 256 * 4 * 4}
NCH = {1: 16, 2: 32, 3: 64, 4: 128, 5: 1}


def _build(taps=()):
    import contextlib
    import concourse.bass as bass
    import concourse.mybir as mybir
    from concourse.tile import TileContext

    F32 = mybir.dt.float32
    BF16 = mybir.dt.bfloat16
    FP8 = mybir.dt.float8e4
    A = mybir.AluOpType
    AF = mybir.ActivationFunctionType
    AX = mybir.AxisListType
    PM = mybir.MatmulPerfMode

    nc = bass.Bass(num_devices=NCORES)

    p1_d = nc.dram_tensor("p1", [128, 16384], BF16, kind="ExternalInput")
    w1rep_d = nc.dram_tensor("w1rep", [128, 16], F32, kind="ExternalInput")
    sw1_d = nc.dram_tensor("sw1", [128, 1], F32, kind="ExternalInput")
    wsc_cols = {2: 128, 3: 192, 4: 1024, 5: 16}
    wsc_d = {l: nc.dram_tensor(f"w{l}sc", [128, wsc_cols[l]], F32, kind="ExternalInput")
             for l in (2, 3, 4, 5)}
    w3c_d = nc.dram_tensor("w3c", [128, 16], F32, kind="ExternalInput")
    swb_d = {l: nc.dram_tensor(f"sw{l}", [128 if l < 5 else 1, 1], F32, kind="ExternalInput")
             for l in (2, 3, 4, 5)}
    gb_d = {l: nc.dram_tensor(f"gb{l}", [NCH[l], 2], F32, kind="ExternalInput")
            for l in (1, 2, 3, 4, 5)}
    out_d = nc.dram_tensor("out", [1, 512], F32, kind="ExternalOutput")
    tap_d = {}
    for t in taps:
        shp = {"raw1": [128, 16384], "raw2": [128, 8192], "raw3": [128, 4096],
               "raw4": [128, 2048], "raw5": [1, 512]}[t]
        tap_d[t] = nc.dram_tensor("tap_" + t, shp, F32 if t == "raw5" else BF16,
                                  kind="ExternalOutput")

    cc_in = {l: nc.dram_tensor(f"cci{l}", [NCH[l], 2], F32, kind="Internal")
             for l in (1, 2, 3, 4, 5)}
    cc_out = {l: nc.dram_tensor(f"cco{l}", [NCH[l], 2], F32, kind="Internal",
                                addr_space="Shared")
              for l in (1, 2, 3, 4, 5)}

    # inline constants
    ohg1 = np.zeros((128, 256), BF)
    bd16 = np.zeros((128, 128), BF)
    for k in range(128):
        ohg1[k, 128 + 16 * (k // 16)] = 2.0
        bd16[k, 16 * (k // 16):16 * (k // 16) + 16] = -1.0
    oh2b = np.zeros((128, 256), BF); oh2b[:, 128] = BF(2.0)
    on32 = np.zeros((128, 256), BF); on32[:, 128:160] = BF(-1.0)
    on64 = np.zeros((128, 256), BF); on64[:, 128:192] = BF(-1.0)
    neg1 = np.full((128, 128), -1.0, BF)
    onl2 = np.zeros((128, 256), BF)
    ohl2 = np.zeros((128, 256), BF)
    for k in range(128):
        par = (k % 32) // 16
        ohl2[k, 128 + 2 * par] = BF(2.0)
        for c in range(32):
            onl2[k, 128 + 4 * c + 2 * par] = BF(-1.0)
    # fp8 DoubleRow one-hots [128, 2, 256]; sign -2 for ACT-produced (negated)
    # planes. ohdr: plain row=slide; oh3c: L3-dt2 8-co packing row = slide +
    # 8*(2*(p//32)+i); ohl2d: L2 row = slide + 2*((p%32)//16)
    F8 = ml_dtypes.float8_e4m3fn
    ohdr = {}
    for sgn, sv in (("p", 2.0), ("m", -2.0)):
        a = np.zeros((128, 2, 256), F8); a[:, :, 128] = F8(sv)
        ohdr["dr" + sgn] = a
        a = np.zeros((128, 2, 256), F8)
        for p in range(128):
            for i in range(2):
                a[p, i, 128 + 8 * (2 * (p // 32) + i)] = F8(sv)
        ohdr["3c" + sgn] = a
        a = np.zeros((128, 2, 256), F8)
        for p in range(128):
            a[p, :, 128 + 2 * ((p % 32) // 16)] = F8(sv)
        ohdr["l2" + sgn] = a
    # sfm folds raw/psum-layout stats to channels; repm broadcasts channel
    # coeffs to PAD-buffer partition layout (channel = p % C for all layers)
    sfm, repm = {}, {}
    for l, C in ((1, 16), (2, 32), (3, 64)):
        m = np.zeros((128, C), np.float32)
        r = np.zeros((C, 128), np.float32)
        for k in range(128):
            m[k, (k // 4) if l == 2 else (k % C)] = 1.0
            r[k % C, k] = 1.0
        sfm[l], repm[l] = m, r
    inl = lambda nm, a: nc.inline_tensor(np.ascontiguousarray(a), name=nm)
    ohg1_t, bd16_t = inl("c_ohg1", ohg1), inl("c_bd16", bd16)
    oh2b_t = inl("c_oh2b", oh2b)
    on32_t = inl("c_on32", on32)
    on64_t, neg1_t = inl("c_on64", on64), inl("c_neg1", neg1)
    onl2_t = inl("c_onl2", onl2)
    ohl2_t = inl("c_ohl2", ohl2)
    ohdr_t = {k: inl("c_oh" + k, v) for k, v in ohdr.items()}
    sf_t = {l: inl(f"c_sf{l}", sfm[l]) for l in sfm}
    rep_t = {l: inl(f"c_rep{l}", repm[l]) for l in repm}

    with TileContext(nc) as tc:
        with contextlib.ExitStack() as ctx:
            cp = ctx.enter_context(tc.tile_pool(name="consts", bufs=1))
            hp = ctx.enter_context(tc.tile_pool(name="hpads", bufs=1))
            sp = ctx.enter_context(tc.tile_pool(name="small", bufs=1))
            scratch = ctx.enter_context(tc.tile_pool(name="scratch", bufs=2))
            ps = ctx.enter_context(tc.tile_pool(name="psum", bufs=6, space="PSUM"))
            ps2 = ctx.enter_context(tc.tile_pool(name="psum2", bufs=2, space="PSUM"))

            def load_const(tag, dram, shape, dtype):
                t = cp.tile(shape, dtype, name=tag, tag=tag)
                nc.sync.dma_start(t[:], dram[:])
                return t

            ohg1_s = load_const("ohg1", ohg1_t, [128, 256], BF16)
            bd16_s = load_const("bd16", bd16_t, [128, 128], BF16)
            oh2b_s = load_const("oh2b", oh2b_t, [128, 256], BF16)
            on32_s = load_const("on32", on32_t, [128, 256], BF16)
            on64_s = load_const("on64", on64_t, [128, 256], BF16)
            neg1_s = load_const("neg1", neg1_t, [128, 128], BF16)
            onl2_s = load_const("onl2", onl2_t, [128, 256], BF16)
            ohl2_s = load_const("ohl2", ohl2_t, [128, 256], BF16)
            ohdr_s = {k: load_const("oh" + k, t, [128, 2, 256], FP8)
                      for k, t in ohdr_t.items()}
            sf_s = {l: load_const(f"sf{l}", sf_t[l], [128, NCH[l]], F32) for l in sfm}
            rep_s = {l: load_const(f"rep{l}", rep_t[l], [NCH[l], 128], F32) for l in repm}
            w1rep_s = load_const("w1rep", w1rep_d, [128, 16], F32)
            sw1_s = load_const("sw1", sw1_d, [128, 1], F32)
            wsc_s = {l: load_const(f"wsc{l}", wsc_d[l], [128, wsc_cols[l]], F32)
                     for l in (2, 3, 4, 5)}
            w3c_s = load_const("w3c", w3c_d, [128, 16], F32)
            swb_s = {l: load_const(f"swb{l}", swb_d[l], [128 if l < 5 else 1, 1], F32)
                     for l in (2, 3, 4, 5)}
            gb_s = {l: load_const(f"gb{l}", gb_d[l], [NCH[l], 2], F32)
                    for l in (1, 2, 3, 4, 5)}

            h1p = hp.tile([128, 4, 66, 66], BF16, name="h1p", tag="h1p")
            h2p = hp.tile([128, 8, 34, 34], BF16, name="h2p", tag="h2p")
            h3p = hp.tile([128, 16, 18, 18], BF16, name="h3p", tag="h3p")
            h4p = hp.tile([128, 32, 10, 10], BF16, name="h4p", tag="h4p")
            for t in (h1p, h2p, h3p, h4p):
                nc.gpsimd.memset(t[:], 0.0)

            st_s = {l: sp.tile([128 if l < 5 else 1, LCFG[l]["nst"] if l > 1 else 32],
                               F32, name=f"sts{l}", tag=f"sts{l}") for l in (1, 2, 3, 4, 5)}
            st_q = {l: sp.tile([128 if l < 5 else 1, LCFG[l]["nst"] if l > 1 else 32],
                               F32, name=f"stq{l}", tag=f"stq{l}") for l in (1, 2, 3, 4, 5)}

            def bn_coeffs(l):
                C = NCH[l]
                R = 128 if l < 5 else 1
                stf = sp.tile([R, 2], F32, name=f"stf{l}", tag=f"stf{l}")
                nc.vector.tensor_reduce(stf[:, 0:1], st_s[l][:], AX.X, A.add)
                nc.vector.tensor_reduce(stf[:, 1:2], st_q[l][:], AX.X, A.add)
                stc = sp.tile([C, 2], F32, name=f"stc{l}", tag=f"stc{l}")
                if l in sf_s:
                    psf = ps2.tile([C, 2], F32, name="paux", tag="paux")
                    nc.tensor.matmul(psf[:], sf_s[l][:], stf[:], start=True, stop=True)
                    nc.scalar.copy(stc[:], psf[:])
                else:
                    nc.vector.tensor_copy(stc[:], stf[:])
                nc.sync.dma_start(cc_in[l][:], stc[:])
                nc.gpsimd.collective_compute(
                    "AllReduce", A.add, replica_groups=[list(range(NCORES))],
                    ins=[cc_in[l][:]], outs=[cc_out[l][:]])
                nb = sp.tile([R, 4], F32, name=f"nb{l}", tag=f"nb{l}")
                if l in rep_s:
                    rr = sp.tile([C, 4], F32, name=f"rr{l}", tag=f"rr{l}")
                    nc.sync.dma_start(rr[:, 0:2], cc_out[l][:])
                    nc.vector.tensor_copy(rr[:, 2:4], gb_s[l][:])
                    prr = ps2.tile([128, 4], F32, name="paux", tag="paux")
                    nc.tensor.matmul(prr[:], rep_s[l][:], rr[:], start=True, stop=True)
                    nc.scalar.copy(nb[:], prr[:])
                else:
                    nc.sync.dma_start(nb[:, 0:2], cc_out[l][:])
                    nc.vector.tensor_copy(nb[:, 2:4], gb_s[l][:])
                ic = 1.0 / CNT[l]
                mS = sp.tile([R, 1], F32, name=f"mS{l}", tag=f"mS{l}")
                v = sp.tile([R, 1], F32, name=f"v{l}", tag=f"v{l}")
                nc.vector.tensor_scalar_mul(mS[:], nb[:, 0:1], ic)
                nc.vector.tensor_tensor(v[:], mS[:], mS[:], A.mult)
                mQ = sp.tile([R, 1], F32, name=f"mQ{l}", tag=f"mQ{l}")
                nc.vector.tensor_scalar_mul(mQ[:], nb[:, 1:2], ic)
                nc.vector.tensor_tensor(v[:], mQ[:], v[:], A.subtract)
                nc.vector.tensor_scalar_add(v[:], v[:], EPS)
                y0 = sp.tile([R, 1], F32, name=f"y0{l}", tag=f"y0{l}")
                nc.scalar.activation(y0[:], v[:], AF.Sqrt)
                r0 = sp.tile([R, 1], F32, name=f"r0{l}", tag=f"r0{l}")
                nc.vector.reciprocal(r0[:], y0[:])
                t0 = sp.tile([R, 1], F32, name=f"t0{l}", tag=f"t0{l}")
                nc.vector.tensor_tensor(t0[:], v[:], r0[:], A.mult)
                nc.vector.tensor_tensor(t0[:], y0[:], t0[:], A.add)
                nc.vector.tensor_scalar_mul(t0[:], t0[:], 0.5)
                rsq = sp.tile([R, 1], F32, name=f"rsq{l}", tag=f"rsq{l}")
                nc.vector.reciprocal(rsq[:], t0[:])
                a = sp.tile([R, 1], F32, name=f"a{l}", tag=f"a{l}")
                nc.vector.tensor_tensor(a[:], nb[:, 2:3], rsq[:], A.mult)
                c = sp.tile([R, 1], F32, name=f"c{l}", tag=f"c{l}")
                nc.vector.tensor_tensor(c[:], mS[:], a[:], A.mult)
                nc.vector.tensor_tensor(c[:], nb[:, 3:4], c[:], A.subtract)
                return a, c

            # ---------------- Layer 1 ----------------
            with tc.tile_pool(name="l1raw", bufs=1) as rp1, \
                 tc.tile_pool(name="l1p", bufs=3) as pp1, \
                 tc.tile_pool(name="l1d", bufs=3) as dp1:
                raw1 = rp1.tile([128, 16384], BF16, name="raw1", tag="raw1")
                for ch in range(8):
                    p1c = pp1.tile([128, 2048], BF16, name="p1c", tag="p1c")
                    nc.sync.dma_start(p1c[:], p1_d[:, ch * 2048:(ch + 1) * 2048])
                    pts = [ps.tile([128, 512], F32, name="pmain", tag="pmain") for _ in range(4)]
                    for tt in range(4):
                        nc.tensor.matmul(pts[tt][:], bd16_s[:],
                                         p1c[:, tt * 512:(tt + 1) * 512],
                                         start=True, stop=False)
                    for co in range(16):
                        d1 = dp1.tile([128, 2048], BF16, name="d1", tag="d1")
                        nc.vector.tensor_scalar(d1[:], p1c[:], w1rep_s[:, co:co + 1],
                                                0.0, A.subtract, A.min)
                        for tt in range(4):
                            nc.tensor.matmul(pts[tt][:],
                                             ohg1_s[:, 128 - co:256 - co],
                                             d1[:, tt * 512:(tt + 1) * 512],
                                             start=False, stop=(co == 15))
                    for tt in range(4):
                        t = ch * 4 + tt
                        nc.scalar.activation(raw1[:, t * 512:(t + 1) * 512], pts[tt][:],
                                             AF.Identity, bias=sw1_s[:, 0:1],
                                             accum_out=st_s[1][:, t:t + 1])
                        sq = scratch.tile([128, 512], F32, name="sq1", tag="sq1")
                        nc.scalar.activation(sq[:], raw1[:, t * 512:(t + 1) * 512],
                                             AF.Square, accum_out=st_q[1][:, t:t + 1])
                        # pre-BN pad writes, overlapped with later chunks
                        base = 32 * (t // 8) + 16 * (t % 2)
                        for g in range(8):
                            nc.sync.dma_start(
                                h1p[base:base + 16, (t % 8) // 2,
                                    1 + 8 * g:9 + 8 * g, 1:65],
                                raw1[16 * g:16 * g + 16, t * 512:(t + 1) * 512])

                a1, c1 = bn_coeffs(1)
                nc.scalar.activation(h1p[:, :, 1:65, 1:65], h1p[:, :, 1:65, 1:65],
                                     AF.Prelu, bias=c1[:, 0:1], scale=a1[:, 0:1],
                                     alpha=SLOPE)

            # ---------------- Layers 2-5 ----------------
            def emit_min(eng, dst, src, wcol):
                # dst: fp8 plane [128, npos]. D/P produce min(p-w, 0);
                # A produces relu(w-p) = -min(p-w,0) (pair with -2 one-hot)
                if eng == "D":
                    nc.vector.tensor_scalar(dst, src, wcol, 0.0, A.subtract, A.min)
                elif eng == "P":
                    nc.gpsimd.tensor_scalar(dst, src, wcol, 0.0, A.subtract, A.min)
                else:
                    nc.scalar.activation(dst, src, AF.Relu, bias=wcol, scale=-1.0)

            def emit_copy(eng, dst, src):
                if eng in ("D", "B"):
                    nc.vector.tensor_copy(dst, src)
                elif eng == "P":
                    nc.gpsimd.tensor_copy(dst, src)
                else:
                    nc.scalar.copy(dst, src)

            def run_layer(l, src_pad, raw_t, pool_p, pool_d, pool_db, pad_emit=None):
                cfg = LCFG[l]
                Ci, Co, K, Ho = cfg["Ci"], cfg["Co"], cfg["K"], cfg["Ho"]
                Wo, rowmod, dparts, npg = Ho, cfg["rowmod"], cfg["dparts"], cfg["npg"]
                ck_n = cfg["ck_n"]
                npos = ck_n * Ho * Wo
                nsub = npos // 512
                nblk = 128 // rowmod
                n_pt = max(1, nsub // nblk)
                nfull = sum(1 for d in dparts if d == 128)
                npairs = nfull // 2
                has_tail = (l == 3)
                onesb = {32: on32_s, 64: on64_s, 128: neg1_s}[rowmod]
                unit = l * 5
                for ch in range(cfg["n_chunks"]):
                    ptiles = []
                    for dt in range(nfull):
                        nk = 128 // Ci
                        pt_ = pool_p.tile([128, ck_n, Ho, Wo], BF16, name=f"p{l}_{dt}", tag=f"p{l}_{dt}")
                        for kk in range(nk):
                            khkw = dt * nk + kk
                            kh, kw = khkw // K, khkw % K
                            n0 = ch * ck_n
                            for gg in range(n0 // npg, (n0 + ck_n - 1) // npg + 1):
                                na = max(n0, gg * npg)
                                nb_ = min(n0 + ck_n, (gg + 1) * npg)
                                if na >= nb_:
                                    continue
                                eng = PATL[l][unit % len(PATL[l])]
                                unit += 1
                                emit_copy(
                                    eng,
                                    pt_[kk * Ci:(kk + 1) * Ci, na - n0:nb_ - n0, :, :],
                                    src_pad[Ci * gg:Ci * gg + Ci,
                                            na - gg * npg:nb_ - gg * npg,
                                            kh:kh + 2 * Ho - 1:2,
                                            kw:kw + 2 * Wo - 1:2])
                        ptiles.append(pt_)
                    pt3 = None
                    if has_tail:
                        # tail dtile (kh=kw=2, 32 d) replicated to 4 groups
                        pt3 = pool_p.tile([128, ck_n, Ho, Wo], BF16,
                                          name=f"p{l}_t", tag=f"p{l}_t")
                        gg = ch  # npg == ck_n == 8 for L3
                        for rep in range(4):
                            eng = PATL[l][unit % len(PATL[l])]
                            unit += 1
                            emit_copy(
                                eng,
                                pt3[rep * 32:rep * 32 + 32, :, :, :],
                                src_pad[32 * gg:32 * gg + 32, :,
                                        2:2 + 2 * Ho - 1:2, 2:2 + 2 * Wo - 1:2])
                    pts = [ps.tile([128, 512], F32, name="pmain", tag="pmain") for _ in range(n_pt)]
                    first = [True] * n_pt
                    for dt in range(nfull):
                        pvf = ptiles[dt][:].rearrange("p a b c -> p (a b c)")
                        for s in range(nsub):
                            tt, j = s // nblk, s % nblk
                            lhs = (onesb[:, :] if rowmod == 128 else
                                   onesb[:, 128 - rowmod * j:256 - rowmod * j])
                            nc.tensor.matmul(pts[tt][:], lhs,
                                             pvf[:, s * 512:(s + 1) * 512],
                                             start=first[tt], stop=False)
                            first[tt] = False
                    if has_tail:
                        pvf = pt3[:].rearrange("p a b c -> p (a b c)")
                        for s in range(nsub):
                            tt, j = s // nblk, s % nblk
                            nc.tensor.matmul(pts[tt][:],
                                             onesb[0:32, 128 - rowmod * j:256 - rowmod * j],
                                             pvf[0:32, s * 512:(s + 1) * 512],
                                             start=False, stop=False)
                    if l == 5:
                        # tiny layer: bf16 (better accuracy, negligible cost)
                        for dt in range(16):
                            dl = pool_d.tile([128, 32, 4, 4], BF16, name="dl5b", tag="dl5b")
                            nc.vector.tensor_scalar(
                                dl[:], ptiles[dt][:],
                                wsc_s[5][:, dt:dt + 1], 0.0, A.subtract, A.min)
                            nc.tensor.matmul(
                                pts[0][:], oh2b_s[:, 128:256],
                                dl[:].rearrange("p a b c -> p (a b c)"),
                                start=False, stop=(dt == 15))
                        bunits, dunits = [], []
                    else:
                        pat = PATL[l]
                        units = [("m", co, pr) for co in range(Co) for pr in range(npairs)]
                        if has_tail:
                            units += [("t", c0, 0) for c0 in range(8)]
                        engs = []
                        for kind, _, _ in units:
                            e = pat[unit % len(pat)]
                            unit += 1
                            if kind == "t" and e == "B":
                                e = "D"
                            engs.append(e)
                        bunits = [u for u, e in zip(units, engs) if e == "B"]
                        dunits = [(u, e) for u, e in zip(units, engs) if e != "B"]
                    # pass 1: bf16 units (normal matmuls). All emitted before any
                    # DoubleRow matmul: the PE must not alternate perf modes
                    # within a psum accumulation group (corrupts results).
                    for kind, co, pr in bunits:
                        dl = pool_db.tile([128, 2, npos], BF16,
                                          name=f"dlb{l}", tag=f"dlb{l}")
                        for i in (0, 1):
                            dt = 2 * pr + i
                            emit_min("D", dl[:, i, :],
                                     ptiles[dt][:].rearrange("p a b c -> p (a b c)"),
                                     wsc_s[l][:, dt * Co + co:dt * Co + co + 1])
                        for s in range(nsub):
                            tt, j = s // nblk, s % nblk
                            r = (rowmod * j + co) if rowmod < 128 else co
                            for i in (0, 1):
                                nc.tensor.matmul(
                                    pts[tt][:], oh2b_s[:, 128 - r:256 - r],
                                    dl[:, i, s * 512:(s + 1) * 512],
                                    start=False, stop=False)
                    # pass 2: fp8 DoubleRow units
                    for ui, ((kind, a0, a1), eng) in enumerate(dunits):
                        lastu = (ui == len(dunits) - 1)
                        dl = pool_d.tile([128, 2, npos], FP8, name=f"dl{l}", tag=f"dl{l}")
                        if kind == "m":
                            co = a0
                            for i in (0, 1):
                                dt = 2 * a1 + i
                                emit_min(eng, dl[:, i, :],
                                         ptiles[dt][:].rearrange("p a b c -> p (a b c)"),
                                         wsc_s[l][:, dt * Co + co:dt * Co + co + 1])
                            oh = ohdr_s["dr" + ("m" if eng == "A" else "p")]
                            rbase = a0
                        else:
                            pvf3 = pt3[:].rearrange("p a b c -> p (a b c)")
                            for i in (0, 1):
                                emit_min(eng, dl[:, i, :], pvf3,
                                         w3c_s[:, i * 8 + a0:i * 8 + a0 + 1])
                            oh = ohdr_s["3c" + ("m" if eng == "A" else "p")]
                            rbase = a0
                        for s in range(nsub):
                            tt, j = s // nblk, s % nblk
                            r = (rowmod * j + rbase) if rowmod < 128 else rbase
                            for half in (0, 1):
                                off = s * 512 + half * 256
                                if Co == 1:
                                    outp = pts[tt][0:1, half * 256:half * 256 + 256]
                                    lhsT = oh[:, :, 128:129]
                                else:
                                    outp = pts[tt][:, half * 256:half * 256 + 256]
                                    lhsT = oh[:, :, 128 - r:256 - r]
                                nc.tensor.matmul(
                                    outp, lhsT, dl[:, :, off:off + 256],
                                    start=False,
                                    stop=(lastu and j == nblk - 1 and half == 1),
                                    perf_mode=PM.DoubleRow)
                    R = 128 if l < 5 else 1
                    for tt in range(n_pt):
                        t = ch * n_pt + tt
                        nc.scalar.activation(
                            raw_t[0:R, t * 512:(t + 1) * 512], pts[tt][0:R, :],
                            AF.Identity, bias=swb_s[l][:, 0:1],
                            accum_out=st_s[l][:, t:t + 1])
                        sq = scratch.tile([R, 512], F32, name=f"sq{l}", tag=f"sq{l}")
                        nc.scalar.activation(sq[:], raw_t[0:R, t * 512:(t + 1) * 512],
                                             AF.Square, accum_out=st_q[l][:, t:t + 1])
                        if pad_emit is not None:
                            pad_emit(t)

            # L2: partitions pack (k4, 2 images, 16 ci); chunk = 4 images
            # (2 dlo pairs); psum rows r = 4co + 2par + hh, tile per dlo
            with tc.tile_pool(name="l2raw", bufs=1) as rp2, \
                 tc.tile_pool(name="l2p", bufs=2) as pp2, \
                 tc.tile_pool(name="l2d", bufs=8) as dp2, \
                 tc.tile_pool(name="l2db", bufs=2) as dp2b:
                raw2 = rp2.tile([128, 8192], BF16, name="raw2", tag="raw2")
                unit = 7
                for ch in range(16):
                    ptiles = []
                    for dt in range(4):
                        pt_ = pp2.tile([128, 32, 32], BF16, name=f"p2_{dt}",
                                       tag=f"p2_{dt}")
                        for k4 in range(4):
                            khkw = dt * 4 + k4
                            kh, kw = khkw // 4, khkw % 4
                            eng = PATL[2][unit % len(PATL[2])]
                            unit += 1
                            emit_copy(
                                eng,
                                pt_[32 * k4:32 * k4 + 32, :, :],
                                h1p[32 * (ch // 4):32 * (ch // 4) + 32, ch % 4,
                                    kh:kh + 63:2, kw:kw + 63:2])
                        ptiles.append(pt_)
                    pt = ps.tile([128, 512], F32, name="pmain", tag="pmain")
                    first = True
                    for dt in range(4):
                        pvf = ptiles[dt][:].rearrange("p a b -> p (a b)")
                        for hh in (0, 1):
                            nc.tensor.matmul(
                                pt[:], onl2_s[:, 128 - hh:256 - hh],
                                pvf[:, 512 * hh:512 * hh + 512],
                                start=first, stop=False)
                            first = False
                    units2 = [(co, pr) for co in range(32) for pr in range(2)]
                    pat2 = PATL[2]
                    engs2 = []
                    for _ in units2:
                        engs2.append(pat2[unit % len(pat2)])
                        unit += 1
                    # pass 1: bf16 units (see run_layer: no perf-mode alternation)
                    for (co, pr), e in zip(units2, engs2):
                        if e != "B":
                            continue
                        dl = dp2b.tile([128, 2, 1024], BF16, name="dlb2", tag="dlb2")
                        for i in (0, 1):
                            dt = 2 * pr + i
                            emit_min("D", dl[:, i, :],
                                     ptiles[dt][:].rearrange("p a b -> p (a b)"),
                                     wsc_s[2][:, dt * 32 + co:dt * 32 + co + 1])
                        for hh in (0, 1):
                            r = 4 * co + hh
                            for i in (0, 1):
                                nc.tensor.matmul(
                                    pt[:], ohl2_s[:, 128 - r:256 - r],
                                    dl[:, i, hh * 512:hh * 512 + 512],
                                    start=False, stop=False)
                    dunits2 = [(u, e) for u, e in zip(units2, engs2) if e != "B"]
                    for ui, ((co, pr), eng) in enumerate(dunits2):
                        lastu = (ui == len(dunits2) - 1)
                        dl = dp2.tile([128, 2, 1024], FP8, name="dl2", tag="dl2")
                        for i in (0, 1):
                            dt = 2 * pr + i
                            emit_min(eng, dl[:, i, :],
                                     ptiles[dt][:].rearrange("p a b -> p (a b)"),
                                     wsc_s[2][:, dt * 32 + co:dt * 32 + co + 1])
                        oh = ohdr_s["l2" + ("m" if eng == "A" else "p")]
                        for hh in (0, 1):
                            r = 4 * co + hh
                            for half in (0, 1):
                                off = hh * 512 + half * 256
                                nc.tensor.matmul(
                                    pt[:, half * 256:half * 256 + 256],
                                    oh[:, :, 128 - r:256 - r],
                                    dl[:, :, off:off + 256],
                                    start=False,
                                    stop=(lastu and hh == 1 and half == 1),
                                    perf_mode=PM.DoubleRow)
                    nc.scalar.activation(raw2[:, ch * 512:(ch + 1) * 512], pt[:],
                                         AF.Identity, bias=swb_s[2][:, 0:1],
                                         accum_out=st_s[2][:, ch:ch + 1])
                    sq = scratch.tile([128, 512], F32, name="sq2", tag="sq2")
                    nc.scalar.activation(sq[:], raw2[:, ch * 512:(ch + 1) * 512],
                                         AF.Square, accum_out=st_q[2][:, ch:ch + 1])
                    for j in range(4):
                        pos0 = ch * 2048 + j * 512
                        n, hh = pos0 // 1024, (pos0 % 1024) // 512
                        nc.sync.dma_start(
                            h2p[32 * (n // 8):32 * (n // 8) + 32, n % 8,
                                1 + 16 * hh:17 + 16 * hh, 1:33],
                            raw2[j:128:4, ch * 512:(ch + 1) * 512])
                a2, c2 = bn_coeffs(2)
                nc.scalar.activation(h2p[:, :, 1:33, 1:33], h2p[:, :, 1:33, 1:33],
                                     AF.Prelu, bias=c2[:, 0:1], scale=a2[:, 0:1],
                                     alpha=SLOPE)

            # L3
            with tc.tile_pool(name="l3raw", bufs=1) as rp3, \
                 tc.tile_pool(name="l3p", bufs=2) as pp3, \
                 tc.tile_pool(name="l3d", bufs=8) as dp3, \
                 tc.tile_pool(name="l3db", bufs=2) as dp3b:
                raw3 = rp3.tile([128, 4096], BF16, name="raw3", tag="raw3")

                def pad3(t):
                    for j in range(2):
                        n = (t * 1024 + j * 512) // 256
                        for i in range(2):
                            nc.sync.dma_start(
                                h3p[64 * (n // 16):64 * (n // 16) + 64,
                                    n % 16 + i, 1:17, 1:17],
                                raw3[64 * j:64 * j + 64,
                                     t * 512 + i * 256:t * 512 + (i + 1) * 256])

                run_layer(3, h2p, raw3, pp3, dp3, dp3b, pad_emit=pad3)
                a3, c3 = bn_coeffs(3)
                nc.scalar.activation(h3p[:, :, 1:17, 1:17], h3p[:, :, 1:17, 1:17],
                                     AF.Prelu, bias=c3[:, 0:1], scale=a3[:, 0:1],
                                     alpha=SLOPE)

            # L4
            with tc.tile_pool(name="l4raw", bufs=1) as rp4, \
                 tc.tile_pool(name="l4p", bufs=1) as pp4, \
                 tc.tile_pool(name="l4d", bufs=8) as dp4, \
                 tc.tile_pool(name="l4db", bufs=2) as dp4b:
                raw4 = rp4.tile([128, 2048], BF16, name="raw4", tag="raw4")

                def pad4(t):
                    for i in range(8):
                        nc.sync.dma_start(
                            h4p[:, 8 * t + i, 1:9, 1:9],
                            raw4[:, t * 512 + i * 64:t * 512 + (i + 1) * 64])

                run_layer(4, h3p, raw4, pp4, dp4, dp4b, pad_emit=pad4)
                a4, c4 = bn_coeffs(4)
                nc.scalar.activation(h4p[:, :, 1:9, 1:9], h4p[:, :, 1:9, 1:9],
                                     AF.Prelu, bias=c4[:, 0:1], scale=a4[:, 0:1],
                                     alpha=SLOPE)

            # L5
            with tc.tile_pool(name="l5raw", bufs=1) as rp5, \
                 tc.tile_pool(name="l5p", bufs=1) as pp5, \
                 tc.tile_pool(name="l5d", bufs=8) as dp5, \
                 tc.tile_pool(name="l5db", bufs=2) as dp5b:
                raw5 = rp5.tile([1, 512], F32, name="raw5", tag="raw5")
                run_layer(5, h4p, raw5, pp5, dp5, dp5b)
                a5, c5 = bn_coeffs(5)
                out5 = sp.tile([1, 512], F32, name="out5", tag="out5")
                nc.scalar.activation(out5[:], raw5[:], AF.Sigmoid,
                                     bias=c5[:, 0:1], scale=a5[:, 0:1])
                if "raw5" in tap_d:
                    nc.sync.dma_start(tap_d["raw5"][:], raw5[:])
                nc.sync.dma_start(out_d[:], out5[:])

    return nc


def _host_prep(inputs):
    x = np.asarray(inputs["x"], np.float32)
    W = {l: np.asarray(inputs[f"W{l}"], np.float32) for l in (1, 2, 3, 4, 5)}
    g = {l: np.asarray(inputs[f"g{l}"], np.float32) for l in (1, 2, 3, 4, 5)}
    b = {l: np.asarray(inputs[f"b{l}"], np.float32) for l in (1, 2, 3, 4, 5)}

    W1f = W[1].reshape(16, 16)
    shared = {
        "w1rep": np.ascontiguousarray(np.tile(W1f.T, (8, 1)), np.float32),
        "sw1": np.ascontiguousarray(np.tile(W1f.sum(1), 8)[:, None], np.float32),
    }
    for l in (2, 3, 4, 5):
        Wd = W[l].transpose(2, 3, 1, 0).reshape(-1, W[l].shape[0])  # [D, Co]
        D, Co = Wd.shape
        if l == 2:
            wsc = np.zeros((128, 4 * 32), np.float32)
            for k in range(128):
                for dt in range(4):
                    khkw = dt * 4 + k // 32
                    wsc[k, dt * 32:(dt + 1) * 32] = Wd[khkw * 16 + (k % 16), :]
            shared["w2sc"] = wsc
        else:
            ndt = len(LCFG[l]["dparts"])
            Wp = np.zeros((ndt * 128, Co), np.float32)
            Wp[:D] = Wd
            shared[f"w{l}sc"] = np.ascontiguousarray(
                Wp.reshape(ndt, 128, Co).transpose(1, 0, 2).reshape(128, ndt * Co))
        if l == 3:
            w3c = np.zeros((128, 16), np.float32)
            for p in range(128):
                for i in range(2):
                    for c0 in range(8):
                        w3c[p, i * 8 + c0] = Wd[256 + p % 32,
                                                c0 + 8 * (2 * (p // 32) + i)]
            shared["w3c"] = w3c
        swl = Wd.sum(0)
        if l < 5:
            idx = (lambda k: k // 4) if l == 2 else (lambda k: k % Co)
            shared[f"sw{l}"] = np.asarray(
                [swl[idx(k)] for k in range(128)], np.float32)[:, None]
        else:
            shared[f"sw{l}"] = np.ascontiguousarray(swl[:, None], np.float32)
    for l in (1, 2, 3, 4, 5):
        shared[f"gb{l}"] = np.ascontiguousarray(
            np.stack([g[l].ravel(), b[l].ravel()], 1), np.float32)

    in_maps = []
    for c in range(NCORES):
        xs = x[c * NPC:(c + 1) * NPC, 0]
        xp = np.pad(xs, ((0, 0), (1, 1), (1, 1)))
        s = xp.strides
        win = np.lib.stride_tricks.as_strided(
            xp, (NPC, 64, 64, 4, 4), (s[0], 2 * s[1], 2 * s[2], s[1], s[2]))
        P1 = win.transpose(3, 4, 0, 1, 2).reshape(16, NPC * 4096)
        p1 = np.ascontiguousarray(
            P1.reshape(16, 32, 8, 512).transpose(2, 0, 1, 3).reshape(128, 16384),
            dtype=BF)
        m = dict(shared)
        m["p1"] = p1
        in_maps.append(m)
    return in_maps


def _run(inputs, taps=(), **kw):
    _install_bir_fix()
    from concourse.bass_utils import run_bass_kernel_spmd
    key = tuple(sorted(taps))
    if key not in _cache:
        _cache[key] = _build(taps)
    in_maps = _host_prep(inputs)
    return run_bass_kernel_spmd(_cache[key], in_maps, list(range(NCORES)), **kw)


def kernel(**inputs):
    res = _run(inputs)
    out = np.zeros((256, 1, 4, 4), np.float32)
    for c in range(NCORES):
        o = np.asarray(res.results[c]["out"], np.float32).reshape(NPC, 4, 4)
        out[c * NPC:(c + 1) * NPC, 0] = o
    return out

